# revision 14
# baseline (speedup 1.0000x reference)
"""BERT self-attention (B=16, T=512, C=768, H=12, D=64) on 8 trn2 NeuronCores.

Data-parallel over batch: each core gets 2 batches. Matmul operands fp16,
fp32 PSUM accumulation. Per core:
  xT     b0 via PE transpose, b1 via DMA XBAR transpose (HWDGE).
  Q^T/K^T [feature, token] (lhsT = W_attn tile), V [token, feature] in
         compact 65-col per-head blocks [V_h | ones] (lhsT = xT tile).
  S^T    = K^T-as-lhsT matmul -> [key, query]; two heads of a pair go to
         separate PSUM banks at row positions 0/64 (row-split tile packing).
  P      = exp(S/8 + mask) on ScalarE, fp16.
  y^T    = lhsT=[V_h | ones] matmul -> unnormalized y^T + row sums in PSUM
         row 64; sums staged (scalar copy) and DMA'd into 32-aligned pair
         rows; one reciprocal_approx_fast per 4 heads; K=2 matmul against a
         0/1 pattern replicates the two recip rows across 128 partitions;
         normalization multiply on GpSimd (Pool).
  out    = y^T-as-lhsT matmul + bias add -> fp16 staged tile, one DMA per
         token tile on the sync HWDGE queue.
Bias replication via 0-stride broadcast DMA; biases shipped f32 from host.
"""

import sys

sys.path.insert(0, "/opt/trn_rl_repo")

from contextlib import ExitStack

import numpy as np

B, T, C = 16, 512, 768
H, D = 12, 64
C3 = 3 * C
N_CORES = 8
BC = B // N_CORES           # batches per core
M = BC * T                  # tokens per core
KT = C // 128               # feature k-tiles (6)
TT = M // 128               # token tiles per core (8)
NQK = 2 * C // 128          # q+k feature n-tiles (12)
VW = H * 65                 # v tile width: per-head [V_h | ones] blocks
SCALE = 1.0 / np.sqrt(D)

_cache = {}


def _build():
    import concourse.bass as bass
    import concourse.tile as tile
    from concourse import bacc, mybir
    from concourse.masks import make_identity
    f32 = mybir.dt.float32
    f16 = mybir.dt.float16
    Exp = mybir.ActivationFunctionType.Exp
    Add = mybir.AluOpType.add

    nc = bacc.Bacc("TRN2", target_bir_lowering=False, debug=False,
                   num_devices=N_CORES)
    x_d = nc.dram_tensor("x", [M, C], f16, kind="ExternalInput").ap()
    mask_d = nc.dram_tensor("mask", [BC, T], f32, kind="ExternalInput").ap()
    wa_d = nc.dram_tensor("w_attn", [C, C3], f16, kind="ExternalInput").ap()
    baqk_d = nc.dram_tensor("ba_qk32", [1, 2 * C], f32,
                            kind="ExternalInput").ap()
    bav_d = nc.dram_tensor("ba_v32", [1, C], f32, kind="ExternalInput").ap()
    wp_d = nc.dram_tensor("w_proj", [C, C], f16, kind="ExternalInput").ap()
    bp_d = nc.dram_tensor("bp32", [1, C], f32, kind="ExternalInput").ap()
    e2_d = nc.dram_tensor("e2pat", [34, 128], f16, kind="ExternalInput").ap()
    out_d = nc.dram_tensor("out", [M, C], f16, kind="ExternalOutput").ap()

    with tile.TileContext(nc) as tc, ExitStack() as ctx:
        pp = ctx.enter_context(tc.tile_pool(name="pp", bufs=1))
        np_ = ctx.enter_context(tc.tile_pool(name="norm", bufs=4))
        ap_ = ctx.enter_context(tc.tile_pool(name="att", bufs=6))
        ps_mm = ctx.enter_context(tc.tile_pool(name="ps_mm", bufs=2, space="PSUM"))

        # ---- small critical DMAs first (scalar HWDGE queue) ----
        ba_qk = pp.tile([128, NQK], f32, tag="ba_qk")
        nc.scalar.dma_start(
            ba_qk[:], baqk_d[0, 0:2 * C].rearrange("(j p) -> p j", p=128))
        mask_sb = pp.tile([128, BC * 4], f32, tag="mask")
        nc.scalar.dma_start(
            mask_sb[:],
            mask_d.rearrange("a b -> (a b)").rearrange("(j p) -> p j", p=128))
        ba_v_rep = pp.tile([128, C], f32, tag="ba_v_rep")
        nc.scalar.dma_start(ba_v_rep[:], bav_d[0:1, :].partition_broadcast(128))
        bp_rep = pp.tile([128, C], f32, tag="bp_rep")
        nc.scalar.dma_start(bp_rep[:], bp_d[0:1, :].partition_broadcast(128))

        wa_t = [pp.tile([128, C3], f16, tag=f"wa{k}", name=f"wa{k}")
                for k in range(KT)]
        xT = [pp.tile([128, M], f16, tag=f"xT{k}", name=f"xT{k}")
              for k in range(KT)]

        def wa_load(q, lo, w, ks):
            for k in ks:
                q.dma_start(wa_t[k][:, lo:lo + w],
                            wa_d[k * 128:(k + 1) * 128, lo:lo + w])

        EV, OD = (0, 2, 4), (1, 3, 5)
        with tc.tile_pool(name="xin", bufs=4) as xin:
            # sync: x b0 first (gates the PE transposes)
            xt_ins = []
            for t in range(4):
                xt_in = xin.tile([128, C], f16, tag="x_in", bufs=4,
                                 name=f"x_in{t}")
                xt_ins.append(xt_in)
                nc.sync.dma_start(xt_in[:], x_d[t * 128:(t + 1) * 128, :])
            # gpsimd SWDGE: V-block first (needed ~15us), then QK tail, wp
            wp_t = [pp.tile([128, C], f16, tag=f"wp{k}", name=f"wp{k}")
                    for k in range(KT)]
            wa_load(nc.gpsimd, 1536, 384, range(KT))
            wa_load(nc.gpsimd, 1920, 384, range(KT))
            wa_load(nc.gpsimd, 1152, 384, range(KT))
            for k in range(KT):
                nc.gpsimd.dma_start(wp_t[k][:], wp_d[k * 128:(k + 1) * 128, :])
            # QK chunks: split each by k-parity across scalar/sync
            for lo in (0, 384, 768):
                wa_load(nc.scalar, lo, 384, EV)
                wa_load(nc.sync, lo, 384, OD)
            # b1 xT via DMA XBAR transpose, 3 on each HWDGE queue
            for k in range(KT):
                q = nc.scalar if k % 2 == 0 else nc.sync
                q.dma_start_transpose(
                    xT[k][:, T:M],
                    x_d[T:M, k * 128:(k + 1) * 128])

            ident = pp.tile([128, 128], f16, tag="ident")
            make_identity(nc, ident[:])
            # b0 transposes on PE
            with tc.tile_pool(name="ps_tr", bufs=4, space="PSUM") as ps_tr:
                for t in range(4):
                    for k in range(KT):
                        ptr = ps_tr.tile([128, 128], f16)
                        nc.tensor.transpose(
                            ptr[:], xt_ins[t][:, k * 128:(k + 1) * 128],
                            ident[:])
                        nc.vector.tensor_copy(
                            xT[k][:, t * 128:(t + 1) * 128], ptr[:])

        # 0/1 replication pattern (rows 0,32: cols 0-63; rows 1,33: 64-127)
        E2 = pp.tile([34, 128], f16, tag="E2")
        nc.scalar.dma_start(E2[:], e2_d[:])

        v_t = [pp.tile([128, VW], f16, tag=f"v{t}", name=f"v{t}")
               for t in range(TT)]
        for t in range(TT):
            nc.gpsimd.memset(
                v_t[t].rearrange("p (h c) -> p h c", c=65)[:, :, 64:65], 1.0)

        qkT = [pp.tile([128, M], f16, tag=f"qk{n}", name=f"qk{n}")
               for n in range(NQK)]
        yT_t = [pp.tile([128, M], f16, tag=f"yT{c}", name=f"yT{c}")
                for c in range(KT)]

        ps_s = ctx.enter_context(tc.tile_pool(name="ps_s", bufs=2, space="PSUM"))
        ps_y = ctx.enter_context(tc.tile_pool(name="ps_y", bufs=2, space="PSUM"))

        def qkv_chain(b, i):
            """i in [0, 20): 12 QK n-tiles then 8 V half-tiles."""
            bcol = b * T
            if i < NQK:
                n = i
                p = ps_mm.tile([128, 512], f32, tag="mm", name=f"mm{b}_{i}")
                for k in range(KT):
                    nc.tensor.matmul(
                        p[:],
                        wa_t[k][:, n * 128:(n + 1) * 128],
                        xT[k][:, bcol:bcol + T],
                        start=(k == 0), stop=(k == KT - 1))
                nc.vector.tensor_scalar_add(
                    qkT[n][:, bcol:bcol + T], p[:], ba_qk[:, n:n + 1])
            else:
                j = i - NQK
                t = b * 4 + j // 2
                lo, w = ((0, 512), (512, 256))[j % 2]
                p = ps_mm.tile([128, 512], f32, tag="mm", name=f"mm{b}_{i}")
                for k in range(KT):
                    nc.tensor.matmul(
                        p[:, :w],
                        xT[k][:, t * 128:(t + 1) * 128],
                        wa_t[k][:, 2 * C + lo:2 * C + lo + w],
                        start=(k == 0), stop=(k == KT - 1))
                h0 = lo // D
                nc.vector.tensor_tensor(
                    out=v_t[t].rearrange("p (h c) -> p h c", c=65)
                        [:, h0:h0 + w // D, 0:64],
                    in0=p[:, :w].rearrange("p (h c) -> p h c", c=D),
                    in1=ba_v_rep[:, lo:lo + w].rearrange(
                        "p (h c) -> p h c", c=D),
                    op=Add)

        pair_tiles = {}
        r_tiles = {}

        def attention_hp(b, hp):
            bcol = b * T
            if hp % 2 == 0:
                rt = np_.tile([34, 512], f32, tag="r_all", bufs=3,
                              name=f"r_all{b}_{hp // 2}")
                nc.gpsimd.memset(rt[:], 1.0)
                r_tiles[(b, hp // 2)] = rt
            pair = np_.tile([128, 512], f16, tag="pair", bufs=8,
                            name=f"pair{b}_{hp}")
            pair_tiles[(b, hp)] = pair
            e_tiles = []
            for kt in range(4):
                ps = ps_s.tile([128, 1024], f32)
                for sub in range(2):
                    r0 = 64 * sub
                    nc.tensor.matmul(
                        ps[:, sub * 512:sub * 512 + 512],
                        qkT[6 + hp][r0:r0 + D,
                                    bcol + kt * 128:bcol + (kt + 1) * 128],
                        qkT[hp][r0:r0 + D, bcol:bcol + T],
                        start=True, stop=True)
                e = ap_.tile([128, 1024], f16, tag="e")
                nc.scalar.activation(
                    e[:], ps[:], Exp,
                    bias=mask_sb[:, b * 4 + kt:b * 4 + kt + 1],
                    scale=float(SCALE))
                e_tiles.append(e)
            for sub in range(2):
                h = 2 * hp + sub
                py = ps_y.tile([128, 512], f32)
                for kt in range(4):
                    nc.tensor.matmul(
                        py[0:65, :],
                        v_t[b * 4 + kt][:, 65 * h:65 * (h + 1)],
                        e_tiles[kt][:, sub * 512:sub * 512 + 512],
                        start=(kt == 0), stop=(kt == 3))
                grp, prow = hp // 2, 32 * (hp % 2) + sub
                rs = np_.tile([65, 512], f32, tag="rstage")
                nc.scalar.copy(rs[64:65, :], py[64:65, :])
                nc.sync.dma_start(
                    r_tiles[(b, grp)][prow:prow + 1, :], rs[64:65, :])
                if sub == 0:
                    nc.vector.tensor_copy(pair[0:64, :], py[0:64, :])
                else:
                    st = np_.tile([64, 512], f16, tag="stage")
                    nc.vector.tensor_copy(st[:], py[0:64, :])
                    nc.sync.dma_start(pair[64:128, :], st[:])

        recip_tiles = {}

        def norm_recip(b, grp):
            recip = np_.tile([34, 512], f32, tag="recip", bufs=3)
            nc.vector.reciprocal_approx_fast(recip[:], r_tiles[(b, grp)][:])
            recip16 = np_.tile([34, 512], f16, tag="recip16", bufs=3)
            nc.vector.tensor_copy(recip16[:], recip[:])
            recip_tiles[(b, grp)] = recip16

        def norm_apply(b, hp):
            """Normalize head-pair hp (k-tile hp) of batch b."""
            bcol = b * T
            recip16 = recip_tiles[(b, hp // 2)]
            r = 32 * (hp % 2)
            rep = ps_y.tile([128, 512], f32, tag="py", name=f"rep{b}_{hp}")
            nc.tensor.matmul(
                rep[:], E2[r:r + 2, :], recip16[r:r + 2, :],
                start=True, stop=True)
            nc.vector.tensor_tensor(
                out=yT_t[hp][:, bcol:bcol + T],
                in0=pair_tiles[(b, hp)][:], in1=rep[:],
                op=mybir.AluOpType.mult)

        pj_part = {}
        ot_tiles = {}

        def proj_chunk(b, i, ks=0, ke=KT, partial=False):
            t = b * 4 + i // 2
            lo, w = ((0, 512), (512, 256))[i % 2]
            p = ps_mm.tile([128, 512], f32, tag="mm", name=f"pj{b}_{i}_{ks}")
            for k in range(ks, ke):
                nc.tensor.matmul(
                    p[:, :w],
                    yT_t[k][:, t * 128:(t + 1) * 128],
                    wp_t[k][:, lo:lo + w],
                    start=(k == ks), stop=(k == ke - 1))
            if partial:
                pt = np_.tile([128, 512], f32, tag="pjpart", bufs=8,
                              name=f"pjpart{i}")
                nc.vector.tensor_tensor(
                    out=pt[:, :w], in0=p[:, :w], in1=bp_rep[:, lo:lo + w],
                    op=Add)
                pj_part[(b, i)] = pt
                return
            if i % 2 == 0:
                ot = np_.tile([128, C], f16, tag="ostage", bufs=3,
                              name=f"ot{b}_{i}")
                ot_tiles[(b, t)] = ot
            else:
                ot = ot_tiles[(b, t)]
            if (b, i) in pj_part:
                nc.vector.tensor_tensor(
                    out=ot[:, lo:lo + w], in0=p[:, :w],
                    in1=pj_part[(b, i)][:, :w], op=Add)
            else:
                nc.vector.tensor_tensor(
                    out=ot[:, lo:lo + w], in0=p[:, :w],
                    in1=bp_rep[:, lo:lo + w], op=Add)
            if i % 2 == 1:
                nc.sync.dma_start(
                    out_d[t * 128:(t + 1) * 128, :], ot[:, :])

        # software-pipelined emission
        CHAIN_ORDER = list(range(9)) + list(range(12, 20)) + [9, 10, 11]
        for i in CHAIN_ORDER:
            qkv_chain(0, i)
        qk1 = iter(CHAIN_ORDER)
        for hp in range(6):
            attention_hp(0, hp)
            if hp % 2 == 1:
                norm_recip(0, hp // 2)
                norm_apply(0, hp - 1)
                if hp >= 3:
                    norm_apply(0, hp - 2)
            for _ in range(4 if hp < 2 else 3):
                i = next(qk1, None)
                if i is not None:
                    qkv_chain(1, i)
        norm_apply(0, 5)
        pj0 = iter(range(8))
        for hp in range(6):
            attention_hp(1, hp)
            if hp % 2 == 1:
                norm_recip(1, hp // 2)
                norm_apply(1, hp - 1)
                if hp >= 3:
                    norm_apply(1, hp - 2)
            if hp == 5:
                for i in range(8):
                    proj_chunk(1, i, 0, 5, partial=True)
            i = next(pj0, None)
            if i is not None:
                proj_chunk(0, i)
        norm_apply(1, 5)
        for i in pj0:
            proj_chunk(0, i)
        for i in range(8):
            proj_chunk(1, i, 5, KT)

    nc.compile()
    return nc


def get_compiled():
    if "nc" not in _cache:
        _cache["nc"] = _build()
    return _cache["nc"]


def make_in_maps(x, attention_mask, W_attn, b_attn, W_proj, b_proj):
    x = np.asarray(x, dtype=np.float32).astype(np.float16)
    mask = np.ascontiguousarray(
        np.asarray(attention_mask, dtype=np.float32)[:, 0, 0, :])
    wa = np.asarray(W_attn, dtype=np.float32).astype(np.float16)
    ba = np.asarray(b_attn, dtype=np.float32)
    baqk = np.ascontiguousarray(ba[0:2 * C].reshape(1, 2 * C))
    bav = np.ascontiguousarray(ba[2 * C:].reshape(1, C))
    wp = np.asarray(W_proj, dtype=np.float32).astype(np.float16)
    bp = np.asarray(b_proj, dtype=np.float32).reshape(1, C)
    e2 = np.zeros((34, 128), dtype=np.float16)
    for r in (0, 32):
        e2[r, 0:64] = 1.0
        e2[r + 1, 64:128] = 1.0
    maps = []
    for i in range(N_CORES):
        maps.append({
            "x": np.ascontiguousarray(x[BC * i:BC * (i + 1)].reshape(M, C)),
            "mask": np.ascontiguousarray(mask[BC * i:BC * (i + 1)]),
            "w_attn": wa, "ba_qk32": baqk, "ba_v32": bav,
            "w_proj": wp, "bp32": bp, "e2pat": e2,
        })
    return maps


def kernel(x, attention_mask, W_attn, b_attn, W_proj, b_proj):
    from concourse.bass_utils import run_bass_kernel_spmd

    nc = get_compiled()
    in_maps = make_in_maps(x, attention_mask, W_attn, b_attn, W_proj, b_proj)
    last_err = None
    for _ in range(3):
        try:
            res = run_bass_kernel_spmd(nc, in_maps, list(range(N_CORES)))
            break
        except Exception as e:  # transient NRT device errors: retry
            last_err = e
    else:
        raise last_err
    out = np.concatenate(
        [res.results[i]["out"].reshape(BC, T, C) for i in range(N_CORES)],
        axis=0)
    return out.astype(np.float32)


# revision 47
# speedup vs baseline: 1.2763x; 1.2763x over previous
"""BERT self-attention (B=16, T=512, C=768, H=12, D=64) on 8 trn2 NeuronCores.

Data-parallel over batch: each core gets 2 batches. Matmul operands fp16,
fp32 PSUM accumulation. Per core:
  xT     b0 via PE transpose, b1 via DMA XBAR transpose (HWDGE).
  Q^T/K^T [feature, token] (lhsT = W_attn tile), V [token, feature] in
         compact 65-col per-head blocks [V_h | ones] (lhsT = xT tile).
  S^T    = K^T-as-lhsT matmul -> [key, query]; two heads of a pair go to
         separate PSUM banks at row positions 0/64 (row-split tile packing).
  P      = exp(S/8 + mask) on ScalarE, fp16.
  y^T    = lhsT=[V_h | ones] matmul -> unnormalized y^T + row sums in PSUM
         row 64; sums staged (scalar copy) and DMA'd into 32-aligned pair
         rows; one reciprocal_approx_fast per 4 heads; K=2 matmul against a
         0/1 pattern replicates the two recip rows across 128 partitions;
         normalization multiply on GpSimd (Pool).
  out    = y^T-as-lhsT matmul + bias add -> fp16 staged tile, one DMA per
         token tile on the sync HWDGE queue.
Bias replication via 0-stride broadcast DMA; biases shipped f32 from host.
"""

import sys

sys.path.insert(0, "/opt/trn_rl_repo")

from contextlib import ExitStack

import numpy as np

B, T, C = 16, 512, 768
H, D = 12, 64
C3 = 3 * C
N_CORES = 8
BC = B // N_CORES           # batches per core
M = BC * T                  # tokens per core
KT = C // 128               # feature k-tiles (6)
TT = M // 128               # token tiles per core (8)
NQK = 2 * C // 128          # q+k feature n-tiles (12)
VW = H * 65                 # v tile width: per-head [V_h | ones] blocks
SCALE = 1.0 / np.sqrt(D)

_cache = {}


def _build():
    import concourse.bass as bass
    import concourse.tile as tile
    from concourse import bacc, mybir
    from concourse.masks import make_identity
    f32 = mybir.dt.float32
    f16 = mybir.dt.float16
    Exp = mybir.ActivationFunctionType.Exp
    Add = mybir.AluOpType.add

    nc = bacc.Bacc("TRN2", target_bir_lowering=False, debug=False,
                   num_devices=N_CORES)
    x_d = nc.dram_tensor("x", [M, C], f16, kind="ExternalInput").ap()
    wa_d = nc.dram_tensor("w_attn", [C, C3], f16, kind="ExternalInput").ap()
    # qm32: cols 0-11 = ba_qk per-partition, cols 12-19 = mask per-partition
    qm_d = nc.dram_tensor("qm32", [128, 20], f32, kind="ExternalInput").ap()
    # bb32: row of [ba_v (768) | b_proj (768)], broadcast to 128 partitions
    bb_d = nc.dram_tensor("bb32", [1, 2 * C], f32, kind="ExternalInput").ap()
    wp_d = nc.dram_tensor("w_proj", [C, C], f16, kind="ExternalInput").ap()
    e2_d = nc.dram_tensor("e2pat", [34, 128], f16, kind="ExternalInput").ap()
    out_d = nc.dram_tensor("out", [M, C], f16, kind="ExternalOutput").ap()

    with tile.TileContext(nc) as tc, ExitStack() as ctx:
        pp = ctx.enter_context(tc.tile_pool(name="pp", bufs=1))
        np_ = ctx.enter_context(tc.tile_pool(name="norm", bufs=4))
        ap_ = ctx.enter_context(tc.tile_pool(name="att", bufs=6))
        ps_mm = ctx.enter_context(tc.tile_pool(name="ps_mm", bufs=2, space="PSUM"))

        # ---- merged small DMAs first (scalar HWDGE queue) ----
        qm = pp.tile([128, 20], f32, tag="qm")
        nc.scalar.dma_start(qm[:], qm_d[:])
        ba_qk = qm[:, 0:NQK]
        mask_sb = qm[:, NQK:NQK + BC * 4]
        bb = pp.tile([128, 2 * C], f32, tag="bb")
        nc.scalar.dma_start(bb[:], bb_d[0:1, :].partition_broadcast(128))
        ba_v_rep = bb[:, 0:C]
        bp_rep = bb[:, C:2 * C]

        wa_all = pp.tile([128, KT, C3], f16, tag="wa_all")
        wa_t = [wa_all[:, k, :] for k in range(KT)]
        xT = [pp.tile([128, M], f16, tag=f"xT{k}", name=f"xT{k}")
              for k in range(KT)]

        def wa_load(q, lo, w, k0, k1):
            q.dma_start(
                wa_all[:, k0:k1, lo:lo + w],
                wa_d[k0 * 128:k1 * 128, lo:lo + w].rearrange(
                    "(k p) c -> p k c", p=128))

        # 0/1 replication pattern (rows 0,32: cols 0-63; rows 1,33: 64-127)
        E2 = pp.tile([34, 128], f16, tag="E2")
        v_t = [pp.tile([128, VW], f16, tag=f"v{t}", name=f"v{t}")
               for t in range(TT)]
        r_keys = [(b, g) for b in range(2) for g in range(3)]
        r_tiles = {k: pp.tile([34, 512], f32, tag=f"r{k[0]}_{k[1]}",
                              name=f"r{k[0]}_{k[1]}") for k in r_keys}

        with tc.tile_pool(name="xin", bufs=1) as xin:
            # x b0 split across both HWDGE queues (gates the PE transposes)
            xall = xin.tile([128, 4 * C], f16, tag="x_in")
            for lo, q in ((0, nc.sync), (2, nc.scalar)):
                q.dma_start(
                    xall[:, lo * C:(lo + 2) * C].rearrange(
                        "p (t c) -> p t c", c=C),
                    x_d[lo * 128:(lo + 2) * 128, :].rearrange(
                        "(t p) c -> p t c", p=128))
            xt_ins = [xall[:, t * C:(t + 1) * C] for t in range(4)]
            # Pool compute first so nothing queues behind SWDGE issues
            ident = pp.tile([128, 128], f16, tag="ident")
            make_identity(nc, ident[:])
            for t in range(TT):
                nc.gpsimd.memset(
                    v_t[t].rearrange("p (h c) -> p h c", c=65)[:, :, 64:65],
                    1.0)
            for key in r_keys:
                nc.gpsimd.memset(r_tiles[key][:], 1.0)
            # QK halves split by k across scalar/sync; V + wp on SWDGE
            wa_load(nc.scalar, 0, 768, 0, 3)
            wa_load(nc.sync, 0, 768, 3, KT)
            wa_load(nc.scalar, 768, 768, 0, 3)
            wa_load(nc.sync, 768, 768, 3, KT)
            wa_load(nc.sync, 1536, 768, 0, KT)
            # b1 xT via DMA XBAR transpose (sync), then wp behind them
            for k in range(KT):
                nc.sync.dma_start_transpose(
                    xT[k][:, T:M],
                    x_d[T:M, k * 128:(k + 1) * 128])
            wp_all = pp.tile([128, KT, C], f16, tag="wp_all")
            wp_t = [wp_all[:, k, :] for k in range(KT)]
            nc.sync.dma_start(
                wp_all[:],
                wp_d[:, :].rearrange("(k p) c -> p k c", p=128))
            nc.scalar.dma_start(E2[:], e2_d[:])
            # b0 transposes on PE
            with tc.tile_pool(name="ps_tr", bufs=4, space="PSUM") as ps_tr:
                for t in range(4):
                    for k in range(KT):
                        ptr = ps_tr.tile([128, 128], f16)
                        nc.tensor.transpose(
                            ptr[:], xt_ins[t][:, k * 128:(k + 1) * 128],
                            ident[:])
                        nc.vector.tensor_copy(
                            xT[k][:, t * 128:(t + 1) * 128], ptr[:])

        qkT = [pp.tile([128, M], f16, tag=f"qk{n}", name=f"qk{n}")
               for n in range(NQK)]
        yT_t = [pp.tile([128, M], f16, tag=f"yT{c}", name=f"yT{c}")
                for c in range(KT)]

        ps_s = ctx.enter_context(tc.tile_pool(name="ps_s", bufs=2, space="PSUM"))
        ps_y = ctx.enter_context(tc.tile_pool(name="ps_y", bufs=2, space="PSUM"))

        def qkv_chain(b, i):
            """i in [0, 20): 12 QK n-tiles then 8 V half-tiles."""
            bcol = b * T
            if i < NQK:
                n = i
                p = ps_mm.tile([128, 512], f32, tag="mm", name=f"mm{b}_{i}")
                for k in range(KT):
                    nc.tensor.matmul(
                        p[:],
                        wa_t[k][:, n * 128:(n + 1) * 128],
                        xT[k][:, bcol:bcol + T],
                        start=(k == 0), stop=(k == KT - 1))
                nc.vector.tensor_scalar_add(
                    qkT[n][:, bcol:bcol + T], p[:], ba_qk[:, n:n + 1])
            else:
                j = i - NQK
                t = b * 4 + j // 2
                lo, w = ((0, 512), (512, 256))[j % 2]
                p = ps_mm.tile([128, 512], f32, tag="mm", name=f"mm{b}_{i}")
                for k in range(KT):
                    nc.tensor.matmul(
                        p[:, :w],
                        xT[k][:, t * 128:(t + 1) * 128],
                        wa_t[k][:, 2 * C + lo:2 * C + lo + w],
                        start=(k == 0), stop=(k == KT - 1))
                h0 = lo // D
                nc.vector.tensor_tensor(
                    out=v_t[t].rearrange("p (h c) -> p h c", c=65)
                        [:, h0:h0 + w // D, 0:64],
                    in0=p[:, :w].rearrange("p (h c) -> p h c", c=D),
                    in1=ba_v_rep[:, lo:lo + w].rearrange(
                        "p (h c) -> p h c", c=D),
                    op=Add)

        pair_tiles = {}

        def attention_hp(b, hp):
            bcol = b * T
            pair = np_.tile([128, 512], f16, tag="pair", bufs=8,
                            name=f"pair{b}_{hp}")
            pair_tiles[(b, hp)] = pair
            e_tiles = []
            for kt in range(4):
                ps = ps_s.tile([128, 1024], f32)
                for sub in range(2):
                    r0 = 64 * sub
                    nc.tensor.matmul(
                        ps[:, sub * 512:sub * 512 + 512],
                        qkT[6 + hp][r0:r0 + D,
                                    bcol + kt * 128:bcol + (kt + 1) * 128],
                        qkT[hp][r0:r0 + D, bcol:bcol + T],
                        start=True, stop=True)
                e = ap_.tile([128, 1024], f16, tag="e")
                nc.scalar.activation(
                    e[:], ps[:], Exp,
                    bias=mask_sb[:, b * 4 + kt:b * 4 + kt + 1],
                    scale=float(SCALE))
                e_tiles.append(e)
            for sub in range(2):
                h = 2 * hp + sub
                py = ps_y.tile([128, 512], f32)
                for kt in range(4):
                    nc.tensor.matmul(
                        py[0:65, :],
                        v_t[b * 4 + kt][:, 65 * h:65 * (h + 1)],
                        e_tiles[kt][:, sub * 512:sub * 512 + 512],
                        start=(kt == 0), stop=(kt == 3))
                grp, prow = hp // 2, 32 * (hp % 2) + sub
                rs = np_.tile([65, 512], f32, tag="rstage")
                if b == 0:
                    nc.scalar.copy(rs[64:65, :], py[64:65, :])
                else:
                    nc.vector.tensor_copy(rs[64:65, :], py[64:65, :])
                nc.sync.dma_start(
                    r_tiles[(b, grp)][prow:prow + 1, :], rs[64:65, :])
                if sub == 0:
                    nc.vector.tensor_copy(pair[0:64, :], py[0:64, :])
                else:
                    st = np_.tile([64, 512], f16, tag="stage")
                    nc.vector.tensor_copy(st[:], py[0:64, :])
                    nc.sync.dma_start(pair[64:128, :], st[:])

        recip_tiles = {}

        def norm_recip(b, hp):
            """Reciprocal for head-pair hp, right after its attention."""
            grp, r = hp // 2, 32 * (hp % 2)
            if hp % 2 == 0:
                recip_tiles[(b, grp)] = (
                    np_.tile([34, 512], f32, tag="recip", bufs=3,
                             name=f"recip{b}_{grp}"),
                    np_.tile([34, 512], f16, tag="recip16", bufs=3,
                             name=f"recip16_{b}_{grp}"))
            rf, r16 = recip_tiles[(b, grp)]
            # full-tile ops from partition 0 (custom DVE op + ACT cast are
            # unreliable at a 32-row base partition on hardware)
            nc.vector.reciprocal_approx_fast(rf[:], r_tiles[(b, grp)][:])
            nc.scalar.copy(r16[:], rf[:])

        def norm_apply(b, hp):
            """Normalize head-pair hp (k-tile hp) of batch b."""
            bcol = b * T
            r16 = recip_tiles[(b, hp // 2)][1]
            r = 32 * (hp % 2)
            rep = ps_y.tile([128, 512], f32, tag="py", name=f"rep{b}_{hp}")
            nc.tensor.matmul(
                rep[:], E2[r:r + 2, :], r16[r:r + 2, :],
                start=True, stop=True)
            nc.vector.tensor_tensor(
                out=yT_t[hp][:, bcol:bcol + T],
                in0=pair_tiles[(b, hp)][:], in1=rep[:],
                op=mybir.AluOpType.mult)

        pj_part = {}
        ot_tiles = {}

        def proj_chunk(b, i, ks=0, ke=KT, partial=False, tail=False):
            t = b * 4 + i // 2
            lo, w = ((0, 512), (512, 256))[i % 2]
            p = ps_mm.tile([128, 512], f32, tag="mm", name=f"pj{b}_{i}_{ks}")
            for k in range(ks, ke):
                nc.tensor.matmul(
                    p[:, :w],
                    yT_t[k][:, t * 128:(t + 1) * 128],
                    wp_t[k][:, lo:lo + w],
                    start=(k == ks), stop=(k == ke - 1))
            if partial:
                pt = np_.tile([128, 512], f32, tag="pjpart", bufs=8,
                              name=f"pjpart{i}")
                nc.vector.tensor_tensor(
                    out=pt[:, :w], in0=p[:, :w], in1=bp_rep[:, lo:lo + w],
                    op=Add)
                pj_part[(b, i)] = pt
                return
            if tail:
                if i % 2 == 0:
                    ot = np_.tile([128, C], f16, tag="otail", bufs=4,
                                  name=f"ott{i}")
                    ot_tiles[(b, t)] = ot
                else:
                    ot = ot_tiles[(b, t)]
                off = lo
            else:
                if i % 4 == 0:
                    ot = np_.tile([128, 2 * C], f16, tag="ostage", bufs=3,
                                  name=f"ot{b}_{i}")
                    ot_tiles[(b, t // 2)] = ot
                else:
                    ot = ot_tiles[(b, t // 2)]
                off = (t % 2) * C + lo
            if (b, i) in pj_part:
                nc.vector.tensor_tensor(
                    out=ot[:, off:off + w], in0=p[:, :w],
                    in1=pj_part[(b, i)][:, :w], op=Add)
            else:
                nc.vector.tensor_tensor(
                    out=ot[:, off:off + w], in0=p[:, :w],
                    in1=bp_rep[:, lo:lo + w], op=Add)
            if tail and i % 2 == 1:
                q = nc.sync if (i // 2) % 2 == 0 else nc.scalar
                q.dma_start(out_d[t * 128:(t + 1) * 128, :], ot[:, :])
            elif not tail and i % 4 == 3:
                t0 = t - 1
                nc.sync.dma_start(
                    out_d[t0 * 128:(t0 + 2) * 128, :].rearrange(
                        "(t p) c -> p t c", p=128),
                    ot[:].rearrange("p (t c) -> p t c", c=C))

        # software-pipelined emission
        CHAIN_ORDER = list(range(9)) + list(range(12, 20)) + [9, 10, 11]
        for i in CHAIN_ORDER:
            qkv_chain(0, i)
        qk1 = iter(CHAIN_ORDER)
        for hp in range(6):
            attention_hp(0, hp)
            norm_recip(0, hp)
            if hp >= 1:
                norm_apply(0, hp - 1)
            for _ in range(4 if hp < 2 else 3):
                i = next(qk1, None)
                if i is not None:
                    qkv_chain(1, i)
        norm_apply(0, 5)
        pj0 = iter(range(8))
        for hp in range(6):
            attention_hp(1, hp)
            norm_recip(1, hp)
            if hp >= 1:
                norm_apply(1, hp - 1)
            if hp == 5:
                norm_apply(1, 5)
                for i in range(8):
                    proj_chunk(1, i, 0, 5, partial=True)
            i = next(pj0, None)
            if i is not None:
                proj_chunk(0, i)
        for i in pj0:
            proj_chunk(0, i)
        for i in range(8):
            proj_chunk(1, i, 5, KT, tail=True)

    nc.compile()
    return nc


def get_compiled():
    if "nc" not in _cache:
        _cache["nc"] = _build()
    return _cache["nc"]


def make_in_maps(x, attention_mask, W_attn, b_attn, W_proj, b_proj):
    x = np.asarray(x, dtype=np.float32).astype(np.float16)
    mask = np.asarray(attention_mask, dtype=np.float32)[:, 0, 0, :]
    wa = np.asarray(W_attn, dtype=np.float32).astype(np.float16)
    ba = np.asarray(b_attn, dtype=np.float32)
    wp = np.asarray(W_proj, dtype=np.float32).astype(np.float16)
    bp = np.asarray(b_proj, dtype=np.float32)
    bb = np.ascontiguousarray(
        np.concatenate([ba[2 * C:], bp]).reshape(1, 2 * C))
    e2 = np.zeros((34, 128), dtype=np.float16)
    for r in (0, 32):
        e2[r, 0:64] = 1.0
        e2[r + 1, 64:128] = 1.0
    maps = []
    for i in range(N_CORES):
        qm = np.zeros((128, 20), dtype=np.float32)
        qm[:, :NQK] = ba[0:2 * C].reshape(NQK, 128).T
        qm[:, NQK:] = mask[BC * i:BC * (i + 1)].reshape(-1).reshape(
            BC * 4, 128).T
        maps.append({
            "x": np.ascontiguousarray(x[BC * i:BC * (i + 1)].reshape(M, C)),
            "qm32": qm, "bb32": bb,
            "w_attn": wa, "w_proj": wp, "e2pat": e2,
        })
    return maps


def kernel(x, attention_mask, W_attn, b_attn, W_proj, b_proj):
    from concourse.bass_utils import run_bass_kernel_spmd

    nc = get_compiled()
    in_maps = make_in_maps(x, attention_mask, W_attn, b_attn, W_proj, b_proj)
    last_err = None
    for _ in range(3):
        try:
            res = run_bass_kernel_spmd(nc, in_maps, list(range(N_CORES)))
            break
        except Exception as e:  # transient NRT device errors: retry
            last_err = e
    else:
        raise last_err
    out = np.concatenate(
        [res.results[i]["out"].reshape(BC, T, C) for i in range(N_CORES)],
        axis=0)
    return out.astype(np.float32)


# revision 51
# speedup vs baseline: 1.3185x; 1.0330x over previous
"""BERT self-attention (B=16, T=512, C=768, H=12, D=64) on 8 trn2 NeuronCores.

Data-parallel over batch: each core gets 2 batches. Matmul operands fp16,
fp32 PSUM accumulation. Per core:
  xT     b0 via PE transpose, b1 via DMA XBAR transpose (HWDGE).
  Q^T/K^T [feature, token] (lhsT = W_attn tile), V [token, feature] in
         compact 65-col per-head blocks [V_h | ones] (lhsT = xT tile).
  S^T    = K^T-as-lhsT matmul -> [key, query]; two heads of a pair go to
         separate PSUM banks at row positions 0/64 (row-split tile packing).
  P      = exp(S/8 + mask) on ScalarE, fp16.
  y^T    = lhsT=[V_h | ones] matmul -> unnormalized y^T + row sums in PSUM
         row 64; sums staged (scalar copy) and DMA'd into 32-aligned pair
         rows; one reciprocal_approx_fast per 4 heads; K=2 matmul against a
         0/1 pattern replicates the two recip rows across 128 partitions;
         normalization multiply on GpSimd (Pool).
  out    = y^T-as-lhsT matmul + bias add -> fp16 staged tile, one DMA per
         token tile on the sync HWDGE queue.
Bias replication via 0-stride broadcast DMA; biases shipped f32 from host.
"""

import sys

sys.path.insert(0, "/opt/trn_rl_repo")

from contextlib import ExitStack

import numpy as np

B, T, C = 16, 512, 768
H, D = 12, 64
C3 = 3 * C
N_CORES = 8
BC = B // N_CORES           # batches per core
M = BC * T                  # tokens per core
KT = C // 128               # feature k-tiles (6)
TT = M // 128               # token tiles per core (8)
NQK = 2 * C // 128          # q+k feature n-tiles (12)
VW = H * 65                 # v tile width: per-head [V_h | ones] blocks
SCALE = 1.0 / np.sqrt(D)

_cache = {}


def _build():
    import concourse.bass as bass
    import concourse.tile as tile
    from concourse import bacc, mybir
    from concourse.masks import make_identity
    f32 = mybir.dt.float32
    f16 = mybir.dt.float16
    Exp = mybir.ActivationFunctionType.Exp
    Add = mybir.AluOpType.add

    nc = bacc.Bacc("TRN2", target_bir_lowering=False, debug=False,
                   num_devices=N_CORES)
    x_d = nc.dram_tensor("x", [M, C], f16, kind="ExternalInput").ap()
    wa_d = nc.dram_tensor("w_attn", [C, C3], f16, kind="ExternalInput").ap()
    # qm32: cols 0-11 = ba_qk per-partition, cols 12-19 = mask per-partition
    qm_d = nc.dram_tensor("qm32", [128, 20], f32, kind="ExternalInput").ap()
    # bb32: row of [ba_v (768) | b_proj (768)], broadcast to 128 partitions
    bb_d = nc.dram_tensor("bb32", [1, 2 * C], f32, kind="ExternalInput").ap()
    wp_d = nc.dram_tensor("w_proj", [C, C], f16, kind="ExternalInput").ap()
    e2_d = nc.dram_tensor("e2pat", [34, 128], f16, kind="ExternalInput").ap()
    out_d = nc.dram_tensor("out", [M, C], f16, kind="ExternalOutput").ap()

    with tile.TileContext(nc) as tc, ExitStack() as ctx:
        pp = ctx.enter_context(tc.tile_pool(name="pp", bufs=1))
        np_ = ctx.enter_context(tc.tile_pool(name="norm", bufs=4))
        ap_ = ctx.enter_context(tc.tile_pool(name="att", bufs=6))
        ps_mm = ctx.enter_context(tc.tile_pool(name="ps_mm", bufs=2, space="PSUM"))

        # ---- merged small DMAs first (scalar HWDGE queue) ----
        qm = pp.tile([128, 20], f32, tag="qm")
        nc.scalar.dma_start(qm[:], qm_d[:])
        ba_qk = qm[:, 0:NQK]
        mask_sb = qm[:, NQK:NQK + BC * 4]
        bb = pp.tile([128, 2 * C], f32, tag="bb")
        ba_v_rep = bb[:, 0:C]
        bp_rep = bb[:, C:2 * C]

        wa_all = pp.tile([128, KT, C3], f16, tag="wa_all")
        wa_t = [wa_all[:, k, :] for k in range(KT)]
        xT = [pp.tile([128, M], f16, tag=f"xT{k}", name=f"xT{k}")
              for k in range(KT)]

        def wa_load(q, lo, w, k0, k1):
            q.dma_start(
                wa_all[:, k0:k1, lo:lo + w],
                wa_d[k0 * 128:k1 * 128, lo:lo + w].rearrange(
                    "(k p) c -> p k c", p=128))

        # 0/1 replication pattern (rows 0,32: cols 0-63; rows 1,33: 64-127)
        E2 = pp.tile([34, 128], f16, tag="E2")
        v_t = [pp.tile([128, VW], f16, tag=f"v{t}", name=f"v{t}")
               for t in range(TT)]
        r_keys = [(b, g) for b in range(2) for g in range(3)]
        r_tiles = {k: pp.tile([34, 512], f32, tag=f"r{k[0]}_{k[1]}",
                              name=f"r{k[0]}_{k[1]}") for k in r_keys}

        with tc.tile_pool(name="xin", bufs=1) as xin:
            # x b0 split across both HWDGE queues (gates the PE transposes)
            xall = xin.tile([128, 4 * C], f16, tag="x_in")
            for lo, q in ((0, nc.sync), (2, nc.scalar)):
                q.dma_start(
                    xall[:, lo * C:(lo + 2) * C].rearrange(
                        "p (t c) -> p t c", c=C),
                    x_d[lo * 128:(lo + 2) * 128, :].rearrange(
                        "(t p) c -> p t c", p=128))
            xt_ins = [xall[:, t * C:(t + 1) * C] for t in range(4)]
            # Pool compute first so nothing queues behind SWDGE issues
            ident = pp.tile([128, 128], f16, tag="ident")
            make_identity(nc, ident[:])
            for t in range(TT):
                nc.gpsimd.memset(
                    v_t[t].rearrange("p (h c) -> p h c", c=65)[:, :, 64:65],
                    1.0)
            for key in r_keys:
                nc.gpsimd.memset(r_tiles[key][:], 1.0)
            # QK halves split by k across scalar/sync
            wa_load(nc.scalar, 0, 768, 0, 3)
            wa_load(nc.sync, 0, 768, 3, KT)
            wa_load(nc.scalar, 768, 768, 0, 3)
            wa_load(nc.sync, 768, 768, 3, KT)
            # broadcast-bias DMA late on scalar (needed at V evac ~18us)
            nc.scalar.dma_start(bb[:], bb_d[0:1, :].partition_broadcast(128))
            wa_load(nc.sync, 1536, 768, 0, KT)
            # b1 xT via DMA XBAR transpose (sync), then wp behind them
            for k in range(KT):
                nc.sync.dma_start_transpose(
                    xT[k][:, T:M],
                    x_d[T:M, k * 128:(k + 1) * 128])
            wp_all = pp.tile([128, KT, C], f16, tag="wp_all")
            wp_t = [wp_all[:, k, :] for k in range(KT)]
            nc.sync.dma_start(
                wp_all[:],
                wp_d[:, :].rearrange("(k p) c -> p k c", p=128))
            nc.scalar.dma_start(E2[:], e2_d[:])
            # b0 transposes on PE
            with tc.tile_pool(name="ps_tr", bufs=4, space="PSUM") as ps_tr:
                for t in range(4):
                    for k in range(KT):
                        ptr = ps_tr.tile([128, 128], f16)
                        nc.tensor.transpose(
                            ptr[:], xt_ins[t][:, k * 128:(k + 1) * 128],
                            ident[:])
                        nc.vector.tensor_copy(
                            xT[k][:, t * 128:(t + 1) * 128], ptr[:])

        qkT = [pp.tile([128, M], f16, tag=f"qk{n}", name=f"qk{n}")
               for n in range(NQK)]
        yT_t = [pp.tile([128, M], f16, tag=f"yT{c}", name=f"yT{c}")
                for c in range(KT)]

        ps_s = ctx.enter_context(tc.tile_pool(name="ps_s", bufs=2, space="PSUM"))
        ps_y = ctx.enter_context(tc.tile_pool(name="ps_y", bufs=2, space="PSUM"))

        def qkv_chain(b, i):
            """i in [0, 20): 12 QK n-tiles then 8 V half-tiles."""
            bcol = b * T
            if i < NQK:
                n = i
                p = ps_mm.tile([128, 512], f32, tag="mm", name=f"mm{b}_{i}")
                for k in range(KT):
                    nc.tensor.matmul(
                        p[:],
                        wa_t[k][:, n * 128:(n + 1) * 128],
                        xT[k][:, bcol:bcol + T],
                        start=(k == 0), stop=(k == KT - 1))
                nc.vector.tensor_scalar_add(
                    qkT[n][:, bcol:bcol + T], p[:], ba_qk[:, n:n + 1])
            else:
                j = i - NQK
                t = b * 4 + j // 2
                lo, w = ((0, 512), (512, 256))[j % 2]
                p = ps_mm.tile([128, 512], f32, tag="mm", name=f"mm{b}_{i}")
                for k in range(KT):
                    nc.tensor.matmul(
                        p[:, :w],
                        xT[k][:, t * 128:(t + 1) * 128],
                        wa_t[k][:, 2 * C + lo:2 * C + lo + w],
                        start=(k == 0), stop=(k == KT - 1))
                h0 = lo // D
                nc.vector.tensor_tensor(
                    out=v_t[t].rearrange("p (h c) -> p h c", c=65)
                        [:, h0:h0 + w // D, 0:64],
                    in0=p[:, :w].rearrange("p (h c) -> p h c", c=D),
                    in1=ba_v_rep[:, lo:lo + w].rearrange(
                        "p (h c) -> p h c", c=D),
                    op=Add)

        pair_tiles = {}

        def attention_hp(b, hp):
            bcol = b * T
            pair = np_.tile([128, 512], f16, tag="pair", bufs=8,
                            name=f"pair{b}_{hp}")
            pair_tiles[(b, hp)] = pair
            e_tiles = []
            for kt in range(4):
                ps = ps_s.tile([128, 1024], f32)
                for sub in range(2):
                    r0 = 64 * sub
                    nc.tensor.matmul(
                        ps[:, sub * 512:sub * 512 + 512],
                        qkT[6 + hp][r0:r0 + D,
                                    bcol + kt * 128:bcol + (kt + 1) * 128],
                        qkT[hp][r0:r0 + D, bcol:bcol + T],
                        start=True, stop=True)
                e = ap_.tile([128, 1024], f16, tag="e")
                nc.scalar.activation(
                    e[:], ps[:], Exp,
                    bias=mask_sb[:, b * 4 + kt:b * 4 + kt + 1],
                    scale=float(SCALE))
                e_tiles.append(e)
            for sub in range(2):
                h = 2 * hp + sub
                py = ps_y.tile([128, 512], f32)
                for kt in range(4):
                    nc.tensor.matmul(
                        py[0:65, :],
                        v_t[b * 4 + kt][:, 65 * h:65 * (h + 1)],
                        e_tiles[kt][:, sub * 512:sub * 512 + 512],
                        start=(kt == 0), stop=(kt == 3))
                grp, prow = hp // 2, 32 * (hp % 2) + sub
                rs = np_.tile([65, 512], f32, tag="rstage")
                if b == 0:
                    nc.scalar.copy(rs[64:65, :], py[64:65, :])
                else:
                    nc.vector.tensor_copy(rs[64:65, :], py[64:65, :])
                nc.sync.dma_start(
                    r_tiles[(b, grp)][prow:prow + 1, :], rs[64:65, :])
                if sub == 0:
                    nc.vector.tensor_copy(pair[0:64, :], py[0:64, :])
                else:
                    st = np_.tile([64, 512], f16, tag="stage")
                    nc.vector.tensor_copy(st[:], py[0:64, :])
                    nc.sync.dma_start(pair[64:128, :], st[:])

        recip_tiles = {}

        def norm_recip(b, hp):
            """Reciprocal for head-pair hp, right after its attention."""
            grp, r = hp // 2, 32 * (hp % 2)
            if hp % 2 == 0:
                recip_tiles[(b, grp)] = (
                    np_.tile([34, 512], f32, tag="recip", bufs=3,
                             name=f"recip{b}_{grp}"),
                    np_.tile([34, 512], f16, tag="recip16", bufs=3,
                             name=f"recip16_{b}_{grp}"))
            rf, r16 = recip_tiles[(b, grp)]
            # full-tile ops from partition 0 (custom DVE op + ACT cast are
            # unreliable at a 32-row base partition on hardware)
            nc.vector.reciprocal_approx_fast(rf[:], r_tiles[(b, grp)][:])
            if b == 0:
                nc.scalar.copy(r16[:], rf[:])
            else:
                nc.vector.tensor_copy(r16[:], rf[:])

        def norm_apply(b, hp):
            """Normalize head-pair hp (k-tile hp) of batch b."""
            bcol = b * T
            r16 = recip_tiles[(b, hp // 2)][1]
            r = 32 * (hp % 2)
            rep = ps_y.tile([128, 512], f32, tag="py", name=f"rep{b}_{hp}")
            nc.tensor.matmul(
                rep[:], E2[r:r + 2, :], r16[r:r + 2, :],
                start=True, stop=True)
            nc.vector.tensor_tensor(
                out=yT_t[hp][:, bcol:bcol + T],
                in0=pair_tiles[(b, hp)][:], in1=rep[:],
                op=mybir.AluOpType.mult)

        pj_part = {}
        ot_tiles = {}

        def proj_chunk(b, i, ks=0, ke=KT, partial=False, tail=False):
            t = b * 4 + i // 2
            lo, w = ((0, 512), (512, 256))[i % 2]
            p = ps_mm.tile([128, 512], f32, tag="mm", name=f"pj{b}_{i}_{ks}")
            for k in range(ks, ke):
                nc.tensor.matmul(
                    p[:, :w],
                    yT_t[k][:, t * 128:(t + 1) * 128],
                    wp_t[k][:, lo:lo + w],
                    start=(k == ks), stop=(k == ke - 1))
            if partial:
                pt = np_.tile([128, 512], f32, tag="pjpart", bufs=8,
                              name=f"pjpart{i}")
                nc.vector.tensor_tensor(
                    out=pt[:, :w], in0=p[:, :w], in1=bp_rep[:, lo:lo + w],
                    op=Add)
                pj_part[(b, i)] = pt
                return
            if tail:
                if i % 2 == 0:
                    ot = np_.tile([128, C], f16, tag="otail", bufs=4,
                                  name=f"ott{i}")
                    ot_tiles[(b, t)] = ot
                else:
                    ot = ot_tiles[(b, t)]
                off = lo
            else:
                if i % 4 == 0:
                    ot = np_.tile([128, 2 * C], f16, tag="ostage", bufs=3,
                                  name=f"ot{b}_{i}")
                    ot_tiles[(b, t // 2)] = ot
                else:
                    ot = ot_tiles[(b, t // 2)]
                off = (t % 2) * C + lo
            if (b, i) in pj_part:
                nc.vector.tensor_tensor(
                    out=ot[:, off:off + w], in0=p[:, :w],
                    in1=pj_part[(b, i)][:, :w], op=Add)
            else:
                nc.vector.tensor_tensor(
                    out=ot[:, off:off + w], in0=p[:, :w],
                    in1=bp_rep[:, lo:lo + w], op=Add)
            if tail and i % 2 == 1:
                q = nc.sync if (i // 2) % 2 == 0 else nc.scalar
                q.dma_start(out_d[t * 128:(t + 1) * 128, :], ot[:, :])
            elif not tail and i % 4 == 3:
                t0 = t - 1
                nc.sync.dma_start(
                    out_d[t0 * 128:(t0 + 2) * 128, :].rearrange(
                        "(t p) c -> p t c", p=128),
                    ot[:].rearrange("p (t c) -> p t c", c=C))

        # software-pipelined emission
        CHAIN_ORDER = list(range(9)) + list(range(12, 20)) + [9, 10, 11]
        for i in CHAIN_ORDER:
            qkv_chain(0, i)
        qk1 = iter(CHAIN_ORDER)
        for hp in range(6):
            attention_hp(0, hp)
            norm_recip(0, hp)
            if hp >= 1:
                norm_apply(0, hp - 1)
            for _ in range(4 if hp < 2 else 3):
                i = next(qk1, None)
                if i is not None:
                    qkv_chain(1, i)
        norm_apply(0, 5)
        pj0 = iter(range(8))
        for hp in range(6):
            attention_hp(1, hp)
            norm_recip(1, hp)
            if hp >= 1:
                norm_apply(1, hp - 1)
            if hp == 5:
                norm_apply(1, 5)
                for i in range(8):
                    proj_chunk(1, i, 0, 5, partial=True)
            for _ in range(1 if hp < 4 else 2):
                i = next(pj0, None)
                if i is not None:
                    proj_chunk(0, i)
        for i in pj0:
            proj_chunk(0, i)
        for i in range(8):
            proj_chunk(1, i, 5, KT, tail=True)

    nc.compile()
    return nc


def get_compiled():
    if "nc" not in _cache:
        _cache["nc"] = _build()
    return _cache["nc"]


def make_in_maps(x, attention_mask, W_attn, b_attn, W_proj, b_proj):
    x = np.asarray(x, dtype=np.float32).astype(np.float16)
    mask = np.asarray(attention_mask, dtype=np.float32)[:, 0, 0, :]
    wa = np.asarray(W_attn, dtype=np.float32).astype(np.float16)
    ba = np.asarray(b_attn, dtype=np.float32)
    wp = np.asarray(W_proj, dtype=np.float32).astype(np.float16)
    bp = np.asarray(b_proj, dtype=np.float32)
    bb = np.ascontiguousarray(
        np.concatenate([ba[2 * C:], bp]).reshape(1, 2 * C))
    e2 = np.zeros((34, 128), dtype=np.float16)
    for r in (0, 32):
        e2[r, 0:64] = 1.0
        e2[r + 1, 64:128] = 1.0
    maps = []
    for i in range(N_CORES):
        qm = np.zeros((128, 20), dtype=np.float32)
        qm[:, :NQK] = ba[0:2 * C].reshape(NQK, 128).T
        qm[:, NQK:] = mask[BC * i:BC * (i + 1)].reshape(-1).reshape(
            BC * 4, 128).T
        maps.append({
            "x": np.ascontiguousarray(x[BC * i:BC * (i + 1)].reshape(M, C)),
            "qm32": qm, "bb32": bb,
            "w_attn": wa, "w_proj": wp, "e2pat": e2,
        })
    return maps


def kernel(x, attention_mask, W_attn, b_attn, W_proj, b_proj):
    from concourse.bass_utils import run_bass_kernel_spmd

    nc = get_compiled()
    in_maps = make_in_maps(x, attention_mask, W_attn, b_attn, W_proj, b_proj)
    last_err = None
    for _ in range(3):
        try:
            res = run_bass_kernel_spmd(nc, in_maps, list(range(N_CORES)))
            break
        except Exception as e:  # transient NRT device errors: retry
            last_err = e
    else:
        raise last_err
    out = np.concatenate(
        [res.results[i]["out"].reshape(BC, T, C) for i in range(N_CORES)],
        axis=0)
    return out.astype(np.float32)
